# revision 14
# baseline (speedup 1.0000x reference)
"""Trainium2 Bass kernel for nn_CropRoi (3D RoI crop + adaptive max pool).

Contract: kernel(**inputs) takes FULL unsharded inputs
  f:         [B=2, C=128, Df=24, Hf=24, Wf=24] float32 feature map
  inputs:    [B, 1, D=96, H=96, W=96] float32 (only shape used)
  proposals: [N=64, 8] float32 (batch, score, center_zyx, side_zyx)
  scale:     scalar int
and returns the FULL output [N, C, 7, 7, 7] float32.

Strategy: shard proposals across the 8 NeuronCores (8 RoIs per core),
grouping by batch index so each core mostly needs ONE batch-half of f.
A core's majority batch-half [C=128, 24,24,24] is loaded with a single
line-rate DMA (128 x 55KB descriptors) and kept in SBUF; per-proposal
pooling reads crops directly out of the resident feature map, so the
many-tiny-descriptor per-crop DMAs of the naive design disappear.
Minority-batch proposals (at most a couple per core) fall back to a
per-proposal crop-slab DMA. The separable adaptive max-pool runs as
tensor_max accumulation chains on the Vector engine.

ONE SPMD Bass program is built with per-proposal geometry baked in;
per-core specialization is tc.If branches on a core-id input, so a
single run_bass_kernel_spmd launch runs all 8 cores concurrently. A
runtime-bounded hardware loop (tc.For_i on a `reps` input) lets test.py
measure steady-state per-exec HW time by slope without re-launch
overhead.
"""

import sys

if "/opt/trn_rl_repo" not in sys.path:
    sys.path.insert(0, "/opt/trn_rl_repo")

import numpy as np

S_OUT = 7
NEG32 = np.float32(np.finfo(np.float32).min)
N_CORES = 8
P_PER_CORE = 8
MAX_REPS = 4096
UNROLL = 8


# ----------------------------------------------------------------- host geometry
def _bins_1d(L):
    """Adaptive-pool windows for length L split into S_OUT bins.

    Returns (starts, widths) relative to the crop origin. Matches the
    reference's floor/ceil bin edges; for L <= 0 all windows are empty.
    """
    i = np.arange(S_OUT, dtype=np.int64)
    starts = (i * L) // S_OUT
    ends = -((-(i + 1) * L) // S_OUT)
    widths = np.maximum(ends - starts, 0)
    return starts.astype(int), widths.astype(int)


def build_geometry(f_shape, proposals, scale):
    """Mirror the reference's crop-bound computation exactly (float32 ops)."""
    B, C, Df, Hf, Wf = f_shape
    maxd = np.array([Df, Hf, Wf], np.int32)
    p = np.asarray(proposals, np.float32)
    center = p[:, 2:5].astype(np.float32)
    side = p[:, 5:8].astype(np.float32)
    c0f = center - side / np.float32(2.0)
    c1f = c0f + side
    sc = np.float32(scale)
    c0 = np.floor(c0f / sc).astype(np.int32)
    c1 = np.ceil(c1f / sc).astype(np.int32)
    c0 = np.maximum(c0, 0)
    c1 = np.minimum(c1, maxd[None, :])
    b = np.clip(p[:, 0].astype(np.int32), 0, B - 1)

    geoms = []
    for n in range(p.shape[0]):
        L = (c1[n] - c0[n]).astype(int)
        g = {
            "b": int(b[n]),
            "orig": [int(v) for v in c0[n]],
            "L": [int(v) for v in L],
            "empty": bool((L <= 0).any()),
        }
        g["zbins"] = _bins_1d(L[0])
        g["ybins"] = _bins_1d(L[1])
        g["xbins"] = _bins_1d(L[2])
        geoms.append(g)
    return geoms


def _vec_cost(g):
    """DVE cycle estimate for one proposal under the optimal stage order
    (width-1 copies weighted low since they route to Act)."""
    if g["empty"]:
        return 401
    import itertools
    L = list(g["L"])
    bins = [g["zbins"], g["ybins"], g["xbins"]]
    todo = [a for a in range(3) if L[a] != S_OUT]
    if not todo:
        return 140          # Act pack-copy only
    best = None
    for perm in itertools.permutations(todo):
        cur = list(L)
        tot = 0
        for a in perm:
            oth = 1
            for d in range(3):
                if d != a:
                    oth *= cur[d]
            for w in bins[a][1]:
                w = int(w)
                if w == 1:
                    tot += (58 + oth) * 0.4      # likely lands on Act
                else:
                    tot += (w - 1) * (58 + oth)
            cur[a] = S_OUT
        if best is None or tot < best:
            best = tot
    return best


def assign_cores(geoms, f_shape):
    """Partition 64 proposals into 8 groups of 8. Each core loads, per batch
    it touches, one contiguous z-slab of that batch-half of f covering all
    its crops. Assignment minimizes a steady-state model: max over cores of
    max(DVE time, per-core solo DMA time), plus the aggregate HBM time.

    Returns cores: list of {"idxs": [...], "slabs": {b: (zlo, zhi)}}."""
    B, C, Df, Hf, Wf = f_shape
    n = len(geoms)
    rowb = C * Hf * Wf * 4              # bytes per z-row of one batch-half
    dve = [_vec_cost(g) / 0.96 for g in geoms]   # ns

    def slabs_of(idxs):
        sl = {}
        for i in idxs:
            g = geoms[i]
            if g["empty"]:
                continue
            zlo, zhi = g["orig"][0], g["orig"][0] + g["L"][0]
            b = g["b"]
            if b in sl:
                sl[b] = (min(sl[b][0], zlo), max(sl[b][1], zhi))
            else:
                sl[b] = (zlo, zhi)
        return sl

    def score(cores):
        tot = 0.0
        worst = 0.0
        for idxs in cores:
            sl = slabs_of(idxs)
            by = sum((zhi - zlo) * rowb for zlo, zhi in sl.values())
            by += P_PER_CORE * C * S_OUT ** 3 * 4      # output writes
            tot += by
            t = max(sum(dve[i] for i in idxs),          # DVE ns
                    by / 1070.0,                        # solo DMA ns
                    len(sl) * 1500.0)                   # per-DMA overhead-ish
            worst = max(worst, t)
        return max(worst, tot / 1700.0)                 # aggregate HBM ns

    # init: group by batch, z-sort within batch, chunk into groups of 8
    order = sorted(range(n), key=lambda i: (geoms[i]["b"],
                                            geoms[i]["orig"][0]))
    cores = [order[k * P_PER_CORE:(k + 1) * P_PER_CORE]
             for k in range(N_CORES)]

    import random
    rnd = random.Random(0)
    best = score(cores)
    for _ in range(4000):
        a, b2 = rnd.randrange(N_CORES), rnd.randrange(N_CORES)
        if a == b2:
            continue
        ia, ib = rnd.randrange(P_PER_CORE), rnd.randrange(P_PER_CORE)
        cores[a][ia], cores[b2][ib] = cores[b2][ib], cores[a][ia]
        s = score(cores)
        if s < best:
            best = s
        else:
            cores[a][ia], cores[b2][ib] = cores[b2][ib], cores[a][ia]
    return [{"idxs": c, "slabs": slabs_of(c)} for c in cores]


# ----------------------------------------------------------------- bass program
def _emit_chain(eng, dst, srcs, copy_eng=None):
    """dst = elementwise max over srcs (1..K tensors of equal shape).
    Width-1 chains are plain copies and may go to a different engine."""
    if len(srcs) == 1:
        ce = copy_eng or eng
        if hasattr(ce, "tensor_copy"):
            ce.tensor_copy(dst, srcs[0])
        else:
            ce.copy(dst, srcs[0])          # Act engine: activation copy
    else:
        eng.tensor_max(dst, srcs[0], srcs[1])
        for t in range(2, len(srcs)):
            eng.tensor_max(dst, dst, srcs[t])


def _emit_pool(nc, mybir, wpool, opool, g, src, z0, y0, x0, o_ap, p, acc):
    """Separable adaptive max-pool of the crop at (z0,y0,x0) sized g['L']
    inside `src`. Stage processing order is chosen per proposal to minimize
    DVE work (reduce the axis that shrinks downstream cost most, first).
    Width-1 chains (copies) and final pack-copies are routed greedily to
    DVE or Act based on accumulated engine load in `acc` [dve_ns, act_ns].
    Emits the out-DMA on Act's HWDGE ring."""
    S = S_OUT
    C = 128
    L = list(g["L"])
    bins = [g["zbins"], g["ybins"], g["xbins"]]
    offs = [z0, y0, x0]
    ext = list(L)                       # current extent per axis
    mat = False                         # src is a packed work tile

    def slices_of(axis_val_pairs):
        # build index tuple [C, axis0, axis1, axis2] from (axis -> idx|slice)
        idx = [slice(None)] * 4
        for a, v in axis_val_pairs:
            idx[1 + a] = v
        return tuple(idx)

    def stage_cost(a, extents):
        other = 1
        for d in range(3):
            if d != a:
                other *= extents[d]
        return sum(max(int(w) - 1, 1) for w in bins[a][1]) * other

    todo = [a for a in range(3) if L[a] != S]
    # exact search over stage orders (at most 6 permutations)
    import itertools
    best_order, best_total = list(todo), None
    for perm in itertools.permutations(todo):
        cur = list(L)
        tot = 0
        for a in perm:
            tot += stage_cost(a, cur)
            cur[a] = S
        if best_total is None or tot < best_total:
            best_order, best_total = list(perm), tot
    order = best_order

    tags = {0: "yz", 1: "yy", 2: "yx"}
    for a in order:
        starts, widths = bins[a]
        shape = [C] + [S if d == a else ext[d] for d in range(3)]
        pool = opool if all(shape[1 + d] == S for d in range(3)) else wpool
        tag = "yx" if pool is opool else ("yz" if not mat else "yy")
        newt = pool.tile(shape, mybir.dt.float32, tag=tag)
        for i in range(S):
            w = int(widths[i])
            didx = slices_of([(d, slice(0, shape[1 + d])) for d in range(3)
                              if d != a] + [(a, i)])
            dst = newt[didx]
            srcs = []
            for t in range(int(starts[i]) + 0, int(starts[i]) + w):
                sidx = slices_of(
                    [(d, slice(offs[d], offs[d] + shape[1 + d]))
                     for d in range(3) if d != a] + [(a, offs[a] + t)])
                srcs.append(src[sidx])
            if w == 1:
                # pure copy. Final-stage copies feed only the out-DMA (on
                # Act), so routing them to Act creates no Act->DVE stall;
                # earlier-stage copies stay on DVE to avoid cross-engine
                # handoff latency in the middle of the chain.
                fd = 1
                for d in range(3):
                    if d != a:
                        fd *= shape[1 + d]
                if a == order[-1]:
                    acc[1] += (224 + fd) / 1.2
                    nc.scalar.copy(dst, srcs[0])
                else:
                    acc[0] += (58 + fd) / 0.96
                    nc.vector.tensor_copy(dst, srcs[0])
            else:
                fd = 1
                for d in range(3):
                    if d != a:
                        fd *= shape[1 + d]
                nc.vector.tensor_max(dst, srcs[0], srcs[1])
                acc[0] += 1.35 * (58 + fd) / 0.96
                for t in range(2, w):
                    nc.vector.tensor_max(dst, dst, srcs[t])
                    acc[0] += 1.35 * (58 + fd) / 0.96
        src = newt
        offs = [0, 0, 0]
        ext[a] = S
        mat = True

    if not mat:
        # all three dims were exactly 7: pack straight out of the resident map
        yx = opool.tile([C, S, S, S], mybir.dt.float32, tag="yx")
        sidx = tuple([slice(None)] + [slice(offs[d], offs[d] + S)
                                      for d in range(3)])
        acc[1] += (224 + 343) / 1.2
        nc.scalar.copy(yx[:], src[sidx])
        src = yx
    nc.scalar.dma_start(out=o_ap[p], in_=src[:])


def _emit_core(nc, mybir, pools, core, geoms, f_ap, o_ap):
    """Emit one core's body: z-slab load(s) + 8 pooled proposals."""
    fpool, cpool, wpool, opool = pools
    C = 128
    Hf, Wf = f_ap.shape[3], f_ap.shape[4]
    S = S_OUT

    slabs = core["slabs"]
    fh = {}
    border = list(slabs.keys())
    for b in border:
        zlo, zhi = slabs[b]
        ft = fpool.tile([C, zhi - zlo, Hf, Wf], mybir.dt.float32, tag="fhalf")
        nc.sync.dma_start(out=ft[:], in_=f_ap[b, :, zlo:zhi, :, :])
        fh[b] = ft

    # proposals for the first-loaded slab run while later slabs stream in
    order = sorted(range(len(core["idxs"])),
                   key=lambda j: (border.index(geoms[core["idxs"][j]]["b"])
                                  if not geoms[core["idxs"][j]]["empty"]
                                  and geoms[core["idxs"][j]]["b"] in fh
                                  else 99))
    acc = [0.0, 0.0]                    # accumulated [DVE ns, Act ns]
    for j in order:
        g = geoms[core["idxs"][j]]
        if g["empty"]:
            yx = opool.tile([C, S, S, S], mybir.dt.float32, tag="yx")
            nc.vector.memset(yx[:], float(NEG32))
            nc.scalar.dma_start(out=o_ap[j], in_=yx[:])
            continue
        zc, yc, xc = g["orig"]
        zlo = slabs[g["b"]][0]
        _emit_pool(nc, mybir, wpool, opool, g, fh[g["b"]],
                   zc - zlo, yc, xc, o_ap, j, acc)


def build_program(f_shape, geoms, cores):
    """One SPMD program: 8 tc.If branches, one per core, each processing
    P_PER_CORE proposals inside a runtime-bounded hardware loop."""
    import concourse.bacc as bacc
    import concourse.tile as tile
    import concourse.mybir as mybir

    B, C, Df, Hf, Wf = f_shape
    assert C == 128
    nc = bacc.Bacc("TRN2", target_bir_lowering=False, debug=False, num_devices=1)
    f_ap = nc.dram_tensor("f", [B, C, Df, Hf, Wf], mybir.dt.float32,
                          kind="ExternalInput").ap()
    reps_t = nc.dram_tensor("reps", [1, 1], mybir.dt.uint32, kind="ExternalInput")
    cid_t = nc.dram_tensor("cid", [1, 1], mybir.dt.uint32, kind="ExternalInput")
    o_ap = nc.dram_tensor("o", [P_PER_CORE, C, S_OUT, S_OUT, S_OUT],
                          mybir.dt.float32, kind="ExternalOutput").ap()

    with tile.TileContext(nc) as tc:
        rtmp = nc.alloc_registers("reps_reg", mybir.ALL_ENGINES)
        nc.regs_load(rtmp, reps_t[0:1, 0:1])
        rv = nc.snap(rtmp, donate=True, min_val=1, max_val=MAX_REPS)
        ctmp = nc.alloc_registers("cid_reg", mybir.ALL_ENGINES)
        nc.regs_load(ctmp, cid_t[0:1, 0:1])
        cid = nc.snap(ctmp, donate=True, min_val=0, max_val=N_CORES - 1)
        with tc.tile_pool(name="fpool", bufs=4) as fpool, \
             tc.tile_pool(name="wpool", bufs=3) as wpool, \
             tc.tile_pool(name="opool", bufs=3) as opool:
            pools = (fpool, None, wpool, opool)
            for k in range(N_CORES):
                with tc.If(cid == k):
                    with tc.For_i(0, rv):
                        # unrolled bodies pipeline: body u+1's slab DMA
                        # overlaps body u's compute (the For_i back edge
                        # barriers all engines, so overlap only happens
                        # within one loop body)
                        for _u in range(UNROLL):
                            _emit_core(nc, mybir, pools, cores[k], geoms,
                                       f_ap, o_ap)
    nc.compile()
    return nc


# ----------------------------------------------------------------- entry points
def make_fast_runner(nc, f, ncores=N_CORES):
    """Low-jitter benchmark runner: builds the sharded jit once, keeps the
    (large, constant) f input device-resident, creates donated zero outputs
    on device. Per call only the tiny reps/cid arrays change."""
    import jax
    import jax.numpy as jnp
    from jax.sharding import Mesh, PartitionSpec, NamedSharding
    from jax.experimental.shard_map import shard_map
    import concourse.mybir as mybir
    from concourse.bass2jax import (_bass_exec_p, install_neuronx_cc_hook,
                                    partition_id_tensor)

    install_neuronx_cc_hook()
    partition_name = (nc.partition_id_tensor.name
                      if nc.partition_id_tensor else None)
    in_names, out_names, out_avals = [], [], []
    for alloc in nc.m.functions[0].allocations:
        if not isinstance(alloc, mybir.MemoryLocationSet):
            continue
        name = alloc.memorylocations[0].name
        if alloc.kind == "ExternalInput":
            if name != partition_name:
                in_names.append(name)
        elif alloc.kind == "ExternalOutput":
            out_names.append(name)
            out_avals.append(jax.core.ShapedArray(
                tuple(alloc.tensor_shape), mybir.dt.np(alloc.dtype)))
    n_params = len(in_names)
    all_names = tuple(in_names + out_names +
                      ([partition_name] if partition_name else []))

    def _body(*args):
        operands = list(args)
        if partition_name is not None:
            operands.append(partition_id_tensor())
        outs = _bass_exec_p.bind(
            *operands,
            out_avals=tuple(out_avals),
            in_names=all_names,
            out_names=tuple(out_names),
            lowering_input_output_aliases=(),
            sim_require_finite=True,
            sim_require_nnan=True,
            nc=nc,
        )
        return tuple(outs)

    devices = jax.devices()[:ncores]
    mesh = Mesh(np.asarray(devices), ("core",))
    n_outs = len(out_names)
    sharded = jax.jit(
        shard_map(_body, mesh=mesh,
                  in_specs=(PartitionSpec("core"),) * (n_params + n_outs),
                  out_specs=(PartitionSpec("core"),) * n_outs,
                  check_rep=False),
        donate_argnums=tuple(range(n_params, n_params + n_outs)),
        keep_unused=True,
    )
    sh = NamedSharding(mesh, PartitionSpec("core"))
    oshape = (ncores * P_PER_CORE, 128, S_OUT, S_OUT, S_OUT)
    zeros_fn = jax.jit(lambda: jnp.zeros(oshape, jnp.float32),
                      out_shardings=sh)
    f_dev = jax.device_put(
        np.concatenate([f] * ncores, axis=0), sh)
    cid_dev = jax.device_put(
        np.arange(ncores, dtype=np.uint32).reshape(ncores, 1), sh)

    def run(reps):
        reps_arr = jax.device_put(
            np.full((ncores, 1), reps, np.uint32), sh)
        outs = sharded(f_dev, reps_arr, cid_dev, zeros_fn())
        outs[0].block_until_ready()
        return outs

    return run


def run_program(nc, f, reps=1):
    from concourse.bass_utils import run_bass_kernel_spmd

    in_maps = [
        {"f": f, "reps": np.array([[reps]], np.uint32),
         "cid": np.array([[k]], np.uint32)}
        for k in range(N_CORES)
    ]
    res = run_bass_kernel_spmd(nc, in_maps, core_ids=list(range(N_CORES)))
    return res


def kernel(**inputs):
    f = np.ascontiguousarray(np.asarray(inputs["f"], dtype=np.float32))
    proposals = np.asarray(inputs["proposals"], dtype=np.float32)
    scale = int(np.asarray(inputs["scale"]))
    geoms = build_geometry(f.shape, proposals, scale)
    cores = assign_cores(geoms, f.shape)
    nc = build_program(f.shape, geoms, cores)
    kernel.last_nc = nc      # reused by test.py for benchmarking
    kernel.last_f = f
    res = run_program(nc, f, reps=1)
    out = np.empty((len(geoms), 128, S_OUT, S_OUT, S_OUT), np.float32)
    for k in range(N_CORES):
        part = np.asarray(res.results[k]["o"])
        for j in range(P_PER_CORE):
            out[cores[k]["idxs"][j]] = part[j]
    return out


kernel.last_nc = None
kernel.last_f = None


# revision 16
# speedup vs baseline: 1.0074x; 1.0074x over previous
"""Trainium2 Bass kernel for nn_CropRoi (3D RoI crop + adaptive max pool).

Contract: kernel(**inputs) takes FULL unsharded inputs
  f:         [B=2, C=128, Df=24, Hf=24, Wf=24] float32 feature map
  inputs:    [B, 1, D=96, H=96, W=96] float32 (only shape used)
  proposals: [N=64, 8] float32 (batch, score, center_zyx, side_zyx)
  scale:     scalar int
and returns the FULL output [N, C, 7, 7, 7] float32.

Strategy: shard proposals across the 8 NeuronCores (8 RoIs per core),
grouping by batch index so each core mostly needs ONE batch-half of f.
A core's majority batch-half [C=128, 24,24,24] is loaded with a single
line-rate DMA (128 x 55KB descriptors) and kept in SBUF; per-proposal
pooling reads crops directly out of the resident feature map, so the
many-tiny-descriptor per-crop DMAs of the naive design disappear.
Minority-batch proposals (at most a couple per core) fall back to a
per-proposal crop-slab DMA. The separable adaptive max-pool runs as
tensor_max accumulation chains on the Vector engine.

ONE SPMD Bass program is built with per-proposal geometry baked in;
per-core specialization is tc.If branches on a core-id input, so a
single run_bass_kernel_spmd launch runs all 8 cores concurrently. A
runtime-bounded hardware loop (tc.For_i on a `reps` input) lets test.py
measure steady-state per-exec HW time by slope without re-launch
overhead.
"""

import sys

if "/opt/trn_rl_repo" not in sys.path:
    sys.path.insert(0, "/opt/trn_rl_repo")

import numpy as np

S_OUT = 7
NEG32 = np.float32(np.finfo(np.float32).min)
N_CORES = 8
P_PER_CORE = 8
MAX_REPS = 4096
UNROLL = 8


# ----------------------------------------------------------------- host geometry
def _bins_1d(L):
    """Adaptive-pool windows for length L split into S_OUT bins.

    Returns (starts, widths) relative to the crop origin. Matches the
    reference's floor/ceil bin edges; for L <= 0 all windows are empty.
    """
    i = np.arange(S_OUT, dtype=np.int64)
    starts = (i * L) // S_OUT
    ends = -((-(i + 1) * L) // S_OUT)
    widths = np.maximum(ends - starts, 0)
    return starts.astype(int), widths.astype(int)


def build_geometry(f_shape, proposals, scale):
    """Mirror the reference's crop-bound computation exactly (float32 ops)."""
    B, C, Df, Hf, Wf = f_shape
    maxd = np.array([Df, Hf, Wf], np.int32)
    p = np.asarray(proposals, np.float32)
    center = p[:, 2:5].astype(np.float32)
    side = p[:, 5:8].astype(np.float32)
    c0f = center - side / np.float32(2.0)
    c1f = c0f + side
    sc = np.float32(scale)
    c0 = np.floor(c0f / sc).astype(np.int32)
    c1 = np.ceil(c1f / sc).astype(np.int32)
    c0 = np.maximum(c0, 0)
    c1 = np.minimum(c1, maxd[None, :])
    b = np.clip(p[:, 0].astype(np.int32), 0, B - 1)

    geoms = []
    for n in range(p.shape[0]):
        L = (c1[n] - c0[n]).astype(int)
        g = {
            "b": int(b[n]),
            "orig": [int(v) for v in c0[n]],
            "L": [int(v) for v in L],
            "empty": bool((L <= 0).any()),
        }
        g["zbins"] = _bins_1d(L[0])
        g["ybins"] = _bins_1d(L[1])
        g["xbins"] = _bins_1d(L[2])
        geoms.append(g)
    return geoms


def _vec_cost(g):
    """DVE cycle estimate for one proposal under the optimal stage order
    (width-1 copies weighted low since they route to Act)."""
    if g["empty"]:
        return 401
    import itertools
    L = list(g["L"])
    bins = [g["zbins"], g["ybins"], g["xbins"]]
    todo = [a for a in range(3) if L[a] != S_OUT]
    if not todo:
        return 140          # Act pack-copy only
    best = None
    for perm in itertools.permutations(todo):
        cur = list(L)
        tot = 0
        for a in perm:
            oth = 1
            for d in range(3):
                if d != a:
                    oth *= cur[d]
            for w in bins[a][1]:
                w = int(w)
                if w == 1:
                    tot += (58 + oth) * 0.4      # likely lands on Act
                else:
                    tot += (w - 1) * (58 + oth)
            cur[a] = S_OUT
        if best is None or tot < best:
            best = tot
    return best


def assign_cores(geoms, f_shape):
    """Partition 64 proposals into 8 groups of 8. Each core loads, per batch
    it touches, one contiguous z-slab of that batch-half of f covering all
    its crops. Assignment minimizes a steady-state model: max over cores of
    max(DVE time, per-core solo DMA time), plus the aggregate HBM time.

    Returns cores: list of {"idxs": [...], "slabs": {b: (zlo, zhi)}}."""
    B, C, Df, Hf, Wf = f_shape
    n = len(geoms)
    rowb = C * Hf * Wf * 4              # bytes per z-row of one batch-half
    dve = [_vec_cost(g) / 0.96 for g in geoms]   # ns

    def slabs_of(idxs):
        sl = {}
        for i in idxs:
            g = geoms[i]
            if g["empty"]:
                continue
            zlo, zhi = g["orig"][0], g["orig"][0] + g["L"][0]
            b = g["b"]
            if b in sl:
                sl[b] = (min(sl[b][0], zlo), max(sl[b][1], zhi))
            else:
                sl[b] = (zlo, zhi)
        return sl

    def score(cores):
        tot = 0.0
        worst = 0.0
        for idxs in cores:
            sl = slabs_of(idxs)
            by = sum((zhi - zlo) * rowb for zlo, zhi in sl.values())
            by += P_PER_CORE * C * S_OUT ** 3 * 4      # output writes
            tot += by
            t = max(sum(dve[i] for i in idxs),          # DVE ns
                    by / 1070.0,                        # solo DMA ns
                    len(sl) * 1500.0)                   # per-DMA overhead-ish
            worst = max(worst, t)
        return max(worst, tot / 1700.0)                 # aggregate HBM ns

    # init: group by batch, z-sort within batch, chunk into groups of 8
    order = sorted(range(n), key=lambda i: (geoms[i]["b"],
                                            geoms[i]["orig"][0]))
    cores = [order[k * P_PER_CORE:(k + 1) * P_PER_CORE]
             for k in range(N_CORES)]

    import random
    rnd = random.Random(0)
    best = score(cores)
    for _ in range(4000):
        a, b2 = rnd.randrange(N_CORES), rnd.randrange(N_CORES)
        if a == b2:
            continue
        ia, ib = rnd.randrange(P_PER_CORE), rnd.randrange(P_PER_CORE)
        cores[a][ia], cores[b2][ib] = cores[b2][ib], cores[a][ia]
        s = score(cores)
        if s < best:
            best = s
        else:
            cores[a][ia], cores[b2][ib] = cores[b2][ib], cores[a][ia]
    return [{"idxs": c, "slabs": slabs_of(c)} for c in cores]


# ----------------------------------------------------------------- bass program
def _emit_chain(eng, dst, srcs, copy_eng=None):
    """dst = elementwise max over srcs (1..K tensors of equal shape).
    Width-1 chains are plain copies and may go to a different engine."""
    if len(srcs) == 1:
        ce = copy_eng or eng
        if hasattr(ce, "tensor_copy"):
            ce.tensor_copy(dst, srcs[0])
        else:
            ce.copy(dst, srcs[0])          # Act engine: activation copy
    else:
        eng.tensor_max(dst, srcs[0], srcs[1])
        for t in range(2, len(srcs)):
            eng.tensor_max(dst, dst, srcs[t])


def _emit_pool(nc, mybir, wpool, opool, g, src, z0, y0, x0, o_ap, p, acc):
    """Separable adaptive max-pool of the crop at (z0,y0,x0) sized g['L']
    inside `src`. Stage processing order is chosen per proposal to minimize
    DVE work (reduce the axis that shrinks downstream cost most, first).
    Width-1 chains (copies) and final pack-copies are routed greedily to
    DVE or Act based on accumulated engine load in `acc` [dve_ns, act_ns].
    Emits the out-DMA on Act's HWDGE ring."""
    S = S_OUT
    C = 128
    L = list(g["L"])
    bins = [g["zbins"], g["ybins"], g["xbins"]]
    offs = [z0, y0, x0]
    ext = list(L)                       # current extent per axis
    mat = False                         # src is a packed work tile

    def slices_of(axis_val_pairs):
        # build index tuple [C, axis0, axis1, axis2] from (axis -> idx|slice)
        idx = [slice(None)] * 4
        for a, v in axis_val_pairs:
            idx[1 + a] = v
        return tuple(idx)

    def stage_cost(a, extents):
        other = 1
        for d in range(3):
            if d != a:
                other *= extents[d]
        return sum(max(int(w) - 1, 1) for w in bins[a][1]) * other

    todo = [a for a in range(3) if L[a] != S]
    # exact search over stage orders (at most 6 permutations)
    import itertools
    best_order, best_total = list(todo), None
    for perm in itertools.permutations(todo):
        cur = list(L)
        tot = 0
        for a in perm:
            tot += stage_cost(a, cur)
            cur[a] = S
        if best_total is None or tot < best_total:
            best_order, best_total = list(perm), tot
    order = best_order

    tags = {0: "yz", 1: "yy", 2: "yx"}
    for a in order:
        starts, widths = bins[a]
        shape = [C] + [S if d == a else ext[d] for d in range(3)]
        pool = opool if all(shape[1 + d] == S for d in range(3)) else wpool
        tag = "yx" if pool is opool else ("yz" if not mat else "yy")
        newt = pool.tile(shape, mybir.dt.float32, tag=tag)
        for i in range(S):
            w = int(widths[i])
            didx = slices_of([(d, slice(0, shape[1 + d])) for d in range(3)
                              if d != a] + [(a, i)])
            dst = newt[didx]
            srcs = []
            for t in range(int(starts[i]) + 0, int(starts[i]) + w):
                sidx = slices_of(
                    [(d, slice(offs[d], offs[d] + shape[1 + d]))
                     for d in range(3) if d != a] + [(a, offs[a] + t)])
                srcs.append(src[sidx])
            if w == 1:
                # pure copy. Final-stage copies feed only the out-DMA (on
                # Act), so routing them to Act creates no Act->DVE stall;
                # earlier-stage copies stay on DVE to avoid cross-engine
                # handoff latency in the middle of the chain.
                fd = 1
                for d in range(3):
                    if d != a:
                        fd *= shape[1 + d]
                if a == order[-1]:
                    acc[1] += (224 + fd) / 1.2
                    nc.scalar.copy(dst, srcs[0])
                else:
                    acc[0] += (58 + fd) / 0.96
                    nc.vector.tensor_copy(dst, srcs[0])
            else:
                fd = 1
                for d in range(3):
                    if d != a:
                        fd *= shape[1 + d]
                nc.vector.tensor_max(dst, srcs[0], srcs[1])
                acc[0] += 1.35 * (58 + fd) / 0.96
                for t in range(2, w):
                    nc.vector.tensor_max(dst, dst, srcs[t])
                    acc[0] += 1.35 * (58 + fd) / 0.96
        src = newt
        offs = [0, 0, 0]
        ext[a] = S
        mat = True

    if not mat:
        # all three dims were exactly 7: pack straight out of the resident map
        yx = opool.tile([C, S, S, S], mybir.dt.float32, tag="yx")
        sidx = tuple([slice(None)] + [slice(offs[d], offs[d] + S)
                                      for d in range(3)])
        acc[1] += (224 + 343) / 1.2
        nc.scalar.copy(yx[:], src[sidx])
        src = yx
    nc.scalar.dma_start(out=o_ap[p], in_=src[:])


def _emit_slab_loads(nc, mybir, fpool, core, f_ap):
    """Issue the core's z-slab DMA(s); returns {batch: (tile, zlo)}."""
    C = 128
    Hf, Wf = f_ap.shape[3], f_ap.shape[4]
    fh = {}
    for b, (zlo, zhi) in core["slabs"].items():
        ft = fpool.tile([C, zhi - zlo, Hf, Wf], mybir.dt.float32, tag="fhalf")
        nc.sync.dma_start(out=ft[:], in_=f_ap[b, :, zlo:zhi, :, :])
        fh[b] = (ft, zlo)
    return fh


def _emit_core_body(nc, mybir, wpool, opool, core, geoms, fh, o_ap):
    """Emit one core's compute for all 8 proposals, reading slabs in fh."""
    C = 128
    S = S_OUT
    acc = [0.0, 0.0]                    # accumulated [DVE ns, Act ns]
    for j in range(len(core["idxs"])):
        g = geoms[core["idxs"][j]]
        if g["empty"]:
            yx = opool.tile([C, S, S, S], mybir.dt.float32, tag="yx")
            nc.vector.memset(yx[:], float(NEG32))
            nc.scalar.dma_start(out=o_ap[j], in_=yx[:])
            continue
        zc, yc, xc = g["orig"]
        ft, zlo = fh[g["b"]]
        _emit_pool(nc, mybir, wpool, opool, g, ft,
                   zc - zlo, yc, xc, o_ap, j, acc)


def build_program(f_shape, geoms, cores):
    """One SPMD program: 8 tc.If branches, one per core, each processing
    P_PER_CORE proposals inside a runtime-bounded hardware loop."""
    import concourse.bacc as bacc
    import concourse.tile as tile
    import concourse.mybir as mybir

    B, C, Df, Hf, Wf = f_shape
    assert C == 128
    nc = bacc.Bacc("TRN2", target_bir_lowering=False, debug=False, num_devices=1)
    f_ap = nc.dram_tensor("f", [B, C, Df, Hf, Wf], mybir.dt.float32,
                          kind="ExternalInput").ap()
    reps_t = nc.dram_tensor("reps", [1, 1], mybir.dt.uint32, kind="ExternalInput")
    cid_t = nc.dram_tensor("cid", [1, 1], mybir.dt.uint32, kind="ExternalInput")
    o_ap = nc.dram_tensor("o", [P_PER_CORE, C, S_OUT, S_OUT, S_OUT],
                          mybir.dt.float32, kind="ExternalOutput").ap()

    with tile.TileContext(nc) as tc:
        rtmp = nc.alloc_registers("reps_reg", mybir.ALL_ENGINES)
        nc.regs_load(rtmp, reps_t[0:1, 0:1])
        rv = nc.snap(rtmp, donate=True, min_val=1, max_val=MAX_REPS)
        ctmp = nc.alloc_registers("cid_reg", mybir.ALL_ENGINES)
        nc.regs_load(ctmp, cid_t[0:1, 0:1])
        cid = nc.snap(ctmp, donate=True, min_val=0, max_val=N_CORES - 1)
        with tc.tile_pool(name="fpool", bufs=4) as fpool, \
             tc.tile_pool(name="wpool", bufs=3) as wpool, \
             tc.tile_pool(name="opool", bufs=3) as opool:
            for k in range(N_CORES):
                with tc.If(cid == k):
                    # software-pipelined slab loads inside the loop: body 0
                    # loads its own slab (one exposed DMA per iteration);
                    # while body u computes, body u+1's slab prefetches.
                    # Every load is consumed, so per-iteration HBM traffic
                    # stays exactly UNROLL slab loads.
                    with tc.For_i(0, rv):
                        fh_cur = _emit_slab_loads(nc, mybir, fpool,
                                                  cores[k], f_ap)
                        for _u in range(UNROLL):
                            if _u + 1 < UNROLL:
                                fh_next = _emit_slab_loads(nc, mybir, fpool,
                                                           cores[k], f_ap)
                            _emit_core_body(nc, mybir, wpool, opool,
                                            cores[k], geoms, fh_cur, o_ap)
                            if _u + 1 < UNROLL:
                                fh_cur = fh_next
    nc.compile()
    return nc


# ----------------------------------------------------------------- entry points
def make_fast_runner(nc, f, ncores=N_CORES):
    """Low-jitter benchmark runner: builds the sharded jit once, keeps the
    (large, constant) f input device-resident, creates donated zero outputs
    on device. Per call only the tiny reps/cid arrays change."""
    import jax
    import jax.numpy as jnp
    from jax.sharding import Mesh, PartitionSpec, NamedSharding
    from jax.experimental.shard_map import shard_map
    import concourse.mybir as mybir
    from concourse.bass2jax import (_bass_exec_p, install_neuronx_cc_hook,
                                    partition_id_tensor)

    install_neuronx_cc_hook()
    partition_name = (nc.partition_id_tensor.name
                      if nc.partition_id_tensor else None)
    in_names, out_names, out_avals = [], [], []
    for alloc in nc.m.functions[0].allocations:
        if not isinstance(alloc, mybir.MemoryLocationSet):
            continue
        name = alloc.memorylocations[0].name
        if alloc.kind == "ExternalInput":
            if name != partition_name:
                in_names.append(name)
        elif alloc.kind == "ExternalOutput":
            out_names.append(name)
            out_avals.append(jax.core.ShapedArray(
                tuple(alloc.tensor_shape), mybir.dt.np(alloc.dtype)))
    n_params = len(in_names)
    all_names = tuple(in_names + out_names +
                      ([partition_name] if partition_name else []))

    def _body(*args):
        operands = list(args)
        if partition_name is not None:
            operands.append(partition_id_tensor())
        outs = _bass_exec_p.bind(
            *operands,
            out_avals=tuple(out_avals),
            in_names=all_names,
            out_names=tuple(out_names),
            lowering_input_output_aliases=(),
            sim_require_finite=True,
            sim_require_nnan=True,
            nc=nc,
        )
        return tuple(outs)

    devices = jax.devices()[:ncores]
    mesh = Mesh(np.asarray(devices), ("core",))
    n_outs = len(out_names)
    sharded = jax.jit(
        shard_map(_body, mesh=mesh,
                  in_specs=(PartitionSpec("core"),) * (n_params + n_outs),
                  out_specs=(PartitionSpec("core"),) * n_outs,
                  check_rep=False),
        donate_argnums=tuple(range(n_params, n_params + n_outs)),
        keep_unused=True,
    )
    sh = NamedSharding(mesh, PartitionSpec("core"))
    oshape = (ncores * P_PER_CORE, 128, S_OUT, S_OUT, S_OUT)
    zeros_fn = jax.jit(lambda: jnp.zeros(oshape, jnp.float32),
                      out_shardings=sh)
    f_dev = jax.device_put(
        np.concatenate([f] * ncores, axis=0), sh)
    cid_dev = jax.device_put(
        np.arange(ncores, dtype=np.uint32).reshape(ncores, 1), sh)

    def run(reps):
        reps_arr = jax.device_put(
            np.full((ncores, 1), reps, np.uint32), sh)
        outs = sharded(f_dev, reps_arr, cid_dev, zeros_fn())
        outs[0].block_until_ready()
        return outs

    return run


def run_program(nc, f, reps=1):
    from concourse.bass_utils import run_bass_kernel_spmd

    in_maps = [
        {"f": f, "reps": np.array([[reps]], np.uint32),
         "cid": np.array([[k]], np.uint32)}
        for k in range(N_CORES)
    ]
    res = run_bass_kernel_spmd(nc, in_maps, core_ids=list(range(N_CORES)))
    return res


def kernel(**inputs):
    f = np.ascontiguousarray(np.asarray(inputs["f"], dtype=np.float32))
    proposals = np.asarray(inputs["proposals"], dtype=np.float32)
    scale = int(np.asarray(inputs["scale"]))
    geoms = build_geometry(f.shape, proposals, scale)
    cores = assign_cores(geoms, f.shape)
    nc = build_program(f.shape, geoms, cores)
    kernel.last_nc = nc      # reused by test.py for benchmarking
    kernel.last_f = f
    res = run_program(nc, f, reps=1)
    out = np.empty((len(geoms), 128, S_OUT, S_OUT, S_OUT), np.float32)
    for k in range(N_CORES):
        part = np.asarray(res.results[k]["o"])
        for j in range(P_PER_CORE):
            out[cores[k]["idxs"][j]] = part[j]
    return out


kernel.last_nc = None
kernel.last_f = None


# revision 18
# speedup vs baseline: 1.0087x; 1.0012x over previous
"""Trainium2 Bass kernel for nn_CropRoi (3D RoI crop + adaptive max pool).

Contract: kernel(**inputs) takes FULL unsharded inputs
  f:         [B=2, C=128, Df=24, Hf=24, Wf=24] float32 feature map
  inputs:    [B, 1, D=96, H=96, W=96] float32 (only shape used)
  proposals: [N=64, 8] float32 (batch, score, center_zyx, side_zyx)
  scale:     scalar int
and returns the FULL output [N, C, 7, 7, 7] float32.

Strategy: shard proposals across the 8 NeuronCores (8 RoIs per core),
grouping by batch index so each core mostly needs ONE batch-half of f.
A core's majority batch-half [C=128, 24,24,24] is loaded with a single
line-rate DMA (128 x 55KB descriptors) and kept in SBUF; per-proposal
pooling reads crops directly out of the resident feature map, so the
many-tiny-descriptor per-crop DMAs of the naive design disappear.
Minority-batch proposals (at most a couple per core) fall back to a
per-proposal crop-slab DMA. The separable adaptive max-pool runs as
tensor_max accumulation chains on the Vector engine.

ONE SPMD Bass program is built with per-proposal geometry baked in;
per-core specialization is tc.If branches on a core-id input, so a
single run_bass_kernel_spmd launch runs all 8 cores concurrently. A
runtime-bounded hardware loop (tc.For_i on a `reps` input) lets test.py
measure steady-state per-exec HW time by slope without re-launch
overhead.
"""

import sys

if "/opt/trn_rl_repo" not in sys.path:
    sys.path.insert(0, "/opt/trn_rl_repo")

import numpy as np

S_OUT = 7
NEG32 = np.float32(np.finfo(np.float32).min)
N_CORES = 8
P_PER_CORE = 8
MAX_REPS = 4096
UNROLL = 8


# ----------------------------------------------------------------- host geometry
def _bins_1d(L):
    """Adaptive-pool windows for length L split into S_OUT bins.

    Returns (starts, widths) relative to the crop origin. Matches the
    reference's floor/ceil bin edges; for L <= 0 all windows are empty.
    """
    i = np.arange(S_OUT, dtype=np.int64)
    starts = (i * L) // S_OUT
    ends = -((-(i + 1) * L) // S_OUT)
    widths = np.maximum(ends - starts, 0)
    return starts.astype(int), widths.astype(int)


def build_geometry(f_shape, proposals, scale):
    """Mirror the reference's crop-bound computation exactly (float32 ops)."""
    B, C, Df, Hf, Wf = f_shape
    maxd = np.array([Df, Hf, Wf], np.int32)
    p = np.asarray(proposals, np.float32)
    center = p[:, 2:5].astype(np.float32)
    side = p[:, 5:8].astype(np.float32)
    c0f = center - side / np.float32(2.0)
    c1f = c0f + side
    sc = np.float32(scale)
    c0 = np.floor(c0f / sc).astype(np.int32)
    c1 = np.ceil(c1f / sc).astype(np.int32)
    c0 = np.maximum(c0, 0)
    c1 = np.minimum(c1, maxd[None, :])
    b = np.clip(p[:, 0].astype(np.int32), 0, B - 1)

    geoms = []
    for n in range(p.shape[0]):
        L = (c1[n] - c0[n]).astype(int)
        g = {
            "b": int(b[n]),
            "orig": [int(v) for v in c0[n]],
            "L": [int(v) for v in L],
            "empty": bool((L <= 0).any()),
        }
        g["zbins"] = _bins_1d(L[0])
        g["ybins"] = _bins_1d(L[1])
        g["xbins"] = _bins_1d(L[2])
        geoms.append(g)
    return geoms


def _vec_cost(g):
    """DVE cycle estimate for one proposal under the optimal stage order
    (width-1 copies weighted low since they route to Act)."""
    if g["empty"]:
        return 401
    import itertools
    L = list(g["L"])
    bins = [g["zbins"], g["ybins"], g["xbins"]]
    todo = [a for a in range(3) if L[a] != S_OUT]
    if not todo:
        return 140          # Act pack-copy only
    best = None
    for perm in itertools.permutations(todo):
        cur = list(L)
        tot = 0
        for a in perm:
            oth = 1
            for d in range(3):
                if d != a:
                    oth *= cur[d]
            for w in bins[a][1]:
                w = int(w)
                if w == 1:
                    tot += (58 + oth) * 0.4      # likely lands on Act
                else:
                    tot += (w - 1) * (58 + oth)
            cur[a] = S_OUT
        if best is None or tot < best:
            best = tot
    return best


def assign_cores(geoms, f_shape):
    """Partition 64 proposals into 8 groups of 8. Each core loads, per batch
    it touches, one contiguous z-slab of that batch-half of f covering all
    its crops. Assignment minimizes a steady-state model: max over cores of
    max(DVE time, per-core solo DMA time), plus the aggregate HBM time.

    Returns cores: list of {"idxs": [...], "slabs": {b: (zlo, zhi)}}."""
    B, C, Df, Hf, Wf = f_shape
    n = len(geoms)
    rowb = C * Hf * Wf * 4              # bytes per z-row of one batch-half
    dve = [_vec_cost(g) / 0.96 for g in geoms]   # ns

    def slabs_of(idxs):
        sl = {}
        for i in idxs:
            g = geoms[i]
            if g["empty"]:
                continue
            zlo, zhi = g["orig"][0], g["orig"][0] + g["L"][0]
            b = g["b"]
            if b in sl:
                sl[b] = (min(sl[b][0], zlo), max(sl[b][1], zhi))
            else:
                sl[b] = (zlo, zhi)
        return sl

    def score(cores):
        tot = 0.0
        worst = 0.0
        for idxs in cores:
            sl = slabs_of(idxs)
            by = sum((zhi - zlo) * rowb for zlo, zhi in sl.values())
            by += P_PER_CORE * C * S_OUT ** 3 * 4      # output writes
            tot += by
            t = max(sum(dve[i] for i in idxs),          # DVE ns
                    by / 1070.0,                        # solo DMA ns
                    len(sl) * 1500.0)                   # per-DMA overhead-ish
            worst = max(worst, t)
        return max(worst, tot / 1700.0)                 # aggregate HBM ns

    # init: group by batch, z-sort within batch, chunk into groups of 8
    order = sorted(range(n), key=lambda i: (geoms[i]["b"],
                                            geoms[i]["orig"][0]))
    cores = [order[k * P_PER_CORE:(k + 1) * P_PER_CORE]
             for k in range(N_CORES)]

    import random
    rnd = random.Random(0)
    best = score(cores)
    for _ in range(4000):
        a, b2 = rnd.randrange(N_CORES), rnd.randrange(N_CORES)
        if a == b2:
            continue
        ia, ib = rnd.randrange(P_PER_CORE), rnd.randrange(P_PER_CORE)
        cores[a][ia], cores[b2][ib] = cores[b2][ib], cores[a][ia]
        s = score(cores)
        if s < best:
            best = s
        else:
            cores[a][ia], cores[b2][ib] = cores[b2][ib], cores[a][ia]
    return [{"idxs": c, "slabs": slabs_of(c)} for c in cores]


# ----------------------------------------------------------------- bass program
def _emit_chain(eng, dst, srcs, copy_eng=None):
    """dst = elementwise max over srcs (1..K tensors of equal shape).
    Width-1 chains are plain copies and may go to a different engine."""
    if len(srcs) == 1:
        ce = copy_eng or eng
        if hasattr(ce, "tensor_copy"):
            ce.tensor_copy(dst, srcs[0])
        else:
            ce.copy(dst, srcs[0])          # Act engine: activation copy
    else:
        eng.tensor_max(dst, srcs[0], srcs[1])
        for t in range(2, len(srcs)):
            eng.tensor_max(dst, dst, srcs[t])


def _emit_pool(nc, mybir, wpool, opool, g, src, z0, y0, x0, o_ap, p, acc):
    """Separable adaptive max-pool of the crop at (z0,y0,x0) sized g['L']
    inside `src`. Stage processing order is chosen per proposal to minimize
    DVE work (reduce the axis that shrinks downstream cost most, first).
    Width-1 chains (copies) and final pack-copies are routed greedily to
    DVE or Act based on accumulated engine load in `acc` [dve_ns, act_ns].
    Emits the out-DMA on Act's HWDGE ring."""
    S = S_OUT
    C = 128
    L = list(g["L"])
    bins = [g["zbins"], g["ybins"], g["xbins"]]
    offs = [z0, y0, x0]
    ext = list(L)                       # current extent per axis
    mat = False                         # src is a packed work tile

    def slices_of(axis_val_pairs):
        # build index tuple [C, axis0, axis1, axis2] from (axis -> idx|slice)
        idx = [slice(None)] * 4
        for a, v in axis_val_pairs:
            idx[1 + a] = v
        return tuple(idx)

    def stage_cost(a, extents):
        other = 1
        for d in range(3):
            if d != a:
                other *= extents[d]
        return sum(max(int(w) - 1, 1) for w in bins[a][1]) * other

    todo = [a for a in range(3) if L[a] != S]
    # exact search over stage orders (at most 6 permutations)
    import itertools
    best_order, best_total = list(todo), None
    for perm in itertools.permutations(todo):
        cur = list(L)
        tot = 0
        for a in perm:
            tot += stage_cost(a, cur)
            cur[a] = S
        if best_total is None or tot < best_total:
            best_order, best_total = list(perm), tot
    order = best_order

    tags = {0: "yz", 1: "yy", 2: "yx"}
    for a in order:
        starts, widths = bins[a]
        shape = [C] + [S if d == a else ext[d] for d in range(3)]
        pool = opool if all(shape[1 + d] == S for d in range(3)) else wpool
        tag = "yx" if pool is opool else ("yz" if not mat else "yy")
        newt = pool.tile(shape, mybir.dt.float32, tag=tag)
        for i in range(S):
            w = int(widths[i])
            didx = slices_of([(d, slice(0, shape[1 + d])) for d in range(3)
                              if d != a] + [(a, i)])
            dst = newt[didx]
            srcs = []
            for t in range(int(starts[i]) + 0, int(starts[i]) + w):
                sidx = slices_of(
                    [(d, slice(offs[d], offs[d] + shape[1 + d]))
                     for d in range(3) if d != a] + [(a, offs[a] + t)])
                srcs.append(src[sidx])
            if w == 1:
                # pure copy. Final-stage copies feed only the out-DMA (on
                # Act), so routing them to Act creates no Act->DVE stall;
                # earlier-stage copies stay on DVE to avoid cross-engine
                # handoff latency in the middle of the chain.
                fd = 1
                for d in range(3):
                    if d != a:
                        fd *= shape[1 + d]
                if a == order[-1]:
                    acc[1] += (224 + fd) / 1.2
                    nc.scalar.copy(dst, srcs[0])
                else:
                    acc[0] += (58 + fd) / 0.96
                    nc.vector.tensor_copy(dst, srcs[0])
            else:
                fd = 1
                for d in range(3):
                    if d != a:
                        fd *= shape[1 + d]
                nc.vector.tensor_max(dst, srcs[0], srcs[1])
                acc[0] += 1.35 * (58 + fd) / 0.96
                for t in range(2, w):
                    nc.vector.tensor_max(dst, dst, srcs[t])
                    acc[0] += 1.35 * (58 + fd) / 0.96
        src = newt
        offs = [0, 0, 0]
        ext[a] = S
        mat = True

    if not mat:
        # all three dims were exactly 7: pack straight out of the resident map
        yx = opool.tile([C, S, S, S], mybir.dt.float32, tag="yx")
        sidx = tuple([slice(None)] + [slice(offs[d], offs[d] + S)
                                      for d in range(3)])
        acc[1] += (224 + 343) / 1.2
        nc.scalar.copy(yx[:], src[sidx])
        src = yx
    nc.scalar.dma_start(out=o_ap[p], in_=src[:])


def _emit_slab_loads(nc, mybir, fpool, core, f_ap):
    """Issue the core's z-slab DMA(s); returns {batch: (tile, zlo)}."""
    C = 128
    Hf, Wf = f_ap.shape[3], f_ap.shape[4]
    fh = {}
    for b, (zlo, zhi) in core["slabs"].items():
        ft = fpool.tile([C, zhi - zlo, Hf, Wf], mybir.dt.float32, tag="fhalf")
        nc.sync.dma_start(out=ft[:], in_=f_ap[b, :, zlo:zhi, :, :])
        fh[b] = (ft, zlo)
    return fh


def _emit_core_body(nc, mybir, wpool, opool, core, geoms, fh, o_ap):
    """Emit one core's compute for all 8 proposals, reading slabs in fh."""
    C = 128
    S = S_OUT
    acc = [0.0, 0.0]                    # accumulated [DVE ns, Act ns]
    for j in range(len(core["idxs"])):
        g = geoms[core["idxs"][j]]
        if g["empty"]:
            yx = opool.tile([C, S, S, S], mybir.dt.float32, tag="yx")
            nc.vector.memset(yx[:], float(NEG32))
            nc.scalar.dma_start(out=o_ap[j], in_=yx[:])
            continue
        zc, yc, xc = g["orig"]
        ft, zlo = fh[g["b"]]
        _emit_pool(nc, mybir, wpool, opool, g, ft,
                   zc - zlo, yc, xc, o_ap, j, acc)


def build_program(f_shape, geoms, cores):
    """One SPMD program: 8 tc.If branches, one per core, each processing
    P_PER_CORE proposals inside a runtime-bounded hardware loop."""
    import concourse.bacc as bacc
    import concourse.tile as tile
    import concourse.mybir as mybir

    B, C, Df, Hf, Wf = f_shape
    assert C == 128
    nc = bacc.Bacc("TRN2", target_bir_lowering=False, debug=False, num_devices=1)
    f_ap = nc.dram_tensor("f", [B, C, Df, Hf, Wf], mybir.dt.float32,
                          kind="ExternalInput").ap()
    reps_t = nc.dram_tensor("reps", [1, 1], mybir.dt.uint32, kind="ExternalInput")
    cid_t = nc.dram_tensor("cid", [1, 1], mybir.dt.uint32, kind="ExternalInput")
    o_ap = nc.dram_tensor("o", [P_PER_CORE, C, S_OUT, S_OUT, S_OUT],
                          mybir.dt.float32, kind="ExternalOutput").ap()

    with tile.TileContext(nc) as tc:
        rtmp = nc.alloc_registers("reps_reg", mybir.ALL_ENGINES)
        nc.regs_load(rtmp, reps_t[0:1, 0:1])
        rv = nc.snap(rtmp, donate=True, min_val=1, max_val=MAX_REPS)
        ctmp = nc.alloc_registers("cid_reg", mybir.ALL_ENGINES)
        nc.regs_load(ctmp, cid_t[0:1, 0:1])
        cid = nc.snap(ctmp, donate=True, min_val=0, max_val=N_CORES - 1)
        with tc.tile_pool(name="fpool", bufs=4) as fpool, \
             tc.tile_pool(name="wpool", bufs=3) as wpool, \
             tc.tile_pool(name="opool", bufs=3) as opool:
            for k in range(N_CORES):
                with tc.If(cid == k):
                    # software-pipelined slab loads inside the loop: body 0
                    # loads its own slab (one exposed DMA per iteration);
                    # while body u computes, body u+1's slab prefetches.
                    # Every load is consumed, so per-iteration HBM traffic
                    # stays exactly UNROLL slab loads.
                    with tc.For_i(0, rv):
                        fh_cur = _emit_slab_loads(nc, mybir, fpool,
                                                  cores[k], f_ap)
                        for _u in range(UNROLL):
                            if _u + 1 < UNROLL:
                                fh_next = _emit_slab_loads(nc, mybir, fpool,
                                                           cores[k], f_ap)
                            _emit_core_body(nc, mybir, wpool, opool,
                                            cores[k], geoms, fh_cur, o_ap)
                            if _u + 1 < UNROLL:
                                fh_cur = fh_next
    nc.compile()
    return nc


# ----------------------------------------------------------------- entry points
def make_fast_runner(nc, f, ncores=N_CORES):
    """Low-jitter benchmark runner: builds the sharded jit once, keeps the
    (large, constant) f input device-resident, creates donated zero outputs
    on device. Per call only the tiny reps/cid arrays change."""
    import jax
    import jax.numpy as jnp
    from jax.sharding import Mesh, PartitionSpec, NamedSharding
    from jax.experimental.shard_map import shard_map
    import concourse.mybir as mybir
    from concourse.bass2jax import (_bass_exec_p, install_neuronx_cc_hook,
                                    partition_id_tensor)

    install_neuronx_cc_hook()
    partition_name = (nc.partition_id_tensor.name
                      if nc.partition_id_tensor else None)
    in_names, out_names, out_avals = [], [], []
    for alloc in nc.m.functions[0].allocations:
        if not isinstance(alloc, mybir.MemoryLocationSet):
            continue
        name = alloc.memorylocations[0].name
        if alloc.kind == "ExternalInput":
            if name != partition_name:
                in_names.append(name)
        elif alloc.kind == "ExternalOutput":
            out_names.append(name)
            out_avals.append(jax.core.ShapedArray(
                tuple(alloc.tensor_shape), mybir.dt.np(alloc.dtype)))
    n_params = len(in_names)
    all_names = tuple(in_names + out_names +
                      ([partition_name] if partition_name else []))

    def _body(*args):
        operands = list(args)
        if partition_name is not None:
            operands.append(partition_id_tensor())
        outs = _bass_exec_p.bind(
            *operands,
            out_avals=tuple(out_avals),
            in_names=all_names,
            out_names=tuple(out_names),
            lowering_input_output_aliases=(),
            sim_require_finite=True,
            sim_require_nnan=True,
            nc=nc,
        )
        return tuple(outs)

    devices = jax.devices()[:ncores]
    mesh = Mesh(np.asarray(devices), ("core",))
    n_outs = len(out_names)
    sharded = jax.jit(
        shard_map(_body, mesh=mesh,
                  in_specs=(PartitionSpec("core"),) * (n_params + n_outs),
                  out_specs=(PartitionSpec("core"),) * n_outs,
                  check_rep=False),
        donate_argnums=tuple(range(n_params, n_params + n_outs)),
        keep_unused=True,
    )
    sh = NamedSharding(mesh, PartitionSpec("core"))
    oshape = (ncores * P_PER_CORE, 128, S_OUT, S_OUT, S_OUT)
    zeros_fn = jax.jit(lambda: jnp.zeros(oshape, jnp.float32),
                      out_shardings=sh)
    f_dev = jax.device_put(
        np.concatenate([f] * ncores, axis=0), sh)
    cid_dev = jax.device_put(
        np.arange(ncores, dtype=np.uint32).reshape(ncores, 1), sh)

    def run(reps):
        reps_arr = jax.device_put(
            np.full((ncores, 1), reps, np.uint32), sh)
        outs = sharded(f_dev, reps_arr, cid_dev, zeros_fn())
        outs[0].block_until_ready()
        return outs

    return run


def run_program(nc, f, reps=1):
    from concourse.bass_utils import run_bass_kernel_spmd

    in_maps = [
        {"f": f, "reps": np.array([[reps]], np.uint32),
         "cid": np.array([[k]], np.uint32)}
        for k in range(N_CORES)
    ]
    res = run_bass_kernel_spmd(nc, in_maps, core_ids=list(range(N_CORES)))
    return res


def kernel(**inputs):
    f = np.ascontiguousarray(np.asarray(inputs["f"], dtype=np.float32))
    proposals = np.asarray(inputs["proposals"], dtype=np.float32)
    scale = int(np.asarray(inputs["scale"]))
    geoms = build_geometry(f.shape, proposals, scale)
    cores = assign_cores(geoms, f.shape)
    nc = build_program(f.shape, geoms, cores)
    kernel.last_nc = nc      # reused by test.py for benchmarking
    kernel.last_f = f
    res = run_program(nc, f, reps=1)
    out = np.empty((len(geoms), 128, S_OUT, S_OUT, S_OUT), np.float32)
    for k in range(N_CORES):
        part = np.asarray(res.results[k]["o"])
        for j in range(P_PER_CORE):
            out[cores[k]["idxs"][j]] = part[j]
    return out


kernel.last_nc = None
kernel.last_f = None
